# revision 1
# baseline (speedup 1.0000x reference)
"""Trainium2 Bass kernel for EuclideanDistLoss.

reference:
    diff = latent1 - latent2                  # [B, D]
    d = sqrt(sum(diff^2, axis=1))             # [B]
    dev = d - CUTOFF
    penalty = where(dev > 0, dev^2, PRESSURE * dev^2)
    return mean(penalty)

Strategy: data-parallel over the batch dim across 8 NeuronCores. Each core
streams its 32768x256 shard of both inputs through SBUF ([128, k*256] tiles,
k rows per partition), computes per-sample sum-of-squares via DVE subtract ->
ACT Square -> DVE grouped 3D reduce, then a short tail computes penalties and
a per-partition partial sum [128,1]. The host sums the 8x128 partials in
float64 and divides by the global batch (the "all-reduce" of the scalar).
Measured at the HBM roofline: ~185-206 us per pass per core vs 186 us
theoretical (67.1 MiB/core at ~360 GB/s); a DMA-only variant is no faster,
so compute is fully overlapped.
"""

import numpy as np

B, D = 262144, 256
N_CORES = 8
P = 128
CUTOFF = 0.1
PRESSURE = 10.0

B_LOCAL = B // N_CORES  # 32768
# default per-tile schedule (rows per partition): bulk of k=4 tiles with a
# tapered end so the serial DMA->sub->square->reduce chain after the last
# transfer is as short as possible.
K_DEFAULT = [4] * 61 + [2] * 4 + [1] * 4
BUFS_DEFAULT = 8
TAIL_UNITS = 12         # columns processed in the post-stream tail (rest hidden)


def build_nc(b_local=B_LOCAL, k=K_DEFAULT, repeat=1, bufs=BUFS_DEFAULT, compute=True,
             interleave=False, dma_group=1, split_queues=False):
    """Build + compile the per-core Bass program (SPMD: same program on all cores).

    repeat>1 re-runs the whole streaming pass over the same data (for
    benchmarking: slope of time vs repeat isolates pure on-device time).
    compute=False builds a DMA-only variant (bandwidth ceiling probe).
    interleave=True expects a single host-interleaved input tensor "latab"
    ([2*b_local, D]; per tile, each partition holds its kt a-rows then its kt
    b-rows) so every tile is ONE contiguous DMA from one sequential stream.
    """
    import concourse.bacc as bacc
    import concourse.tile as tile
    from concourse import mybir

    f32 = mybir.dt.float32
    Alu = mybir.AluOpType
    Act = mybir.ActivationFunctionType

    if isinstance(k, int):
        tile_rows = P * k
        assert b_local % tile_rows == 0
        schedule = [k] * (b_local // tile_rows)
    else:  # explicit per-tile k schedule
        schedule = list(k)
        assert sum(schedule) * P == b_local
    T_units = sum(schedule)  # total k-units (= penalties per partition)

    # split point: columns [0, split) get their penalty math + partial-sum DMA
    # issued while the tapered end of the stream is still in flight; each tile
    # beyond split gets its own penalty chain immediately after its reduce, so
    # the post-stream tail is one tiny chain over the last tile's columns.
    split = max(T_units - TAIL_UNITS, 0) if (compute and repeat == 1) else T_units
    n_out_cols = 2

    nc = bacc.Bacc("TRN2", target_bir_lowering=False, debug=False, num_devices=N_CORES)
    if interleave:
        z = nc.dram_tensor("latab", [2 * b_local, D], f32, kind="ExternalInput").ap()
    else:
        a = nc.dram_tensor("latent1", [b_local, D], f32, kind="ExternalInput").ap()
        b = nc.dram_tensor("latent2", [b_local, D], f32, kind="ExternalInput").ap()
    out = nc.dram_tensor("out", [P, n_out_cols], f32, kind="ExternalOutput").ap()

    with tile.TileContext(nc) as tc:
        with (
            tc.tile_pool(name="pa", bufs=bufs) as pa,
            tc.tile_pool(name="pb", bufs=bufs) as pb,
            tc.tile_pool(name="keep", bufs=1) as keep,
        ):
            n = T_units  # penalties per partition
            ssq = keep.tile([P, n], f32)
            d_ = keep.tile([P, n], f32)
            mask = keep.tile([P, n], f32)  # 1.0 where d < CUTOFF
            fac = keep.tile([P, n], f32)   # 1 + (PRESSURE-1)*mask
            dd = keep.tile([P, n], f32)    # (d - CUTOFF)^2
            pen = keep.tile([P, n], f32)
            psum = keep.tile([P, n_out_cols], f32)
            neg_cut = keep.tile([P, 1], f32)
            nc.vector.memset(neg_cut, -CUTOFF)

            def penalty_ops(c_lo, c_hi, out_col):
                # critical path: Sqrt -> Square (both ACT, one table set) ->
                # mult -> reduce; mask/fac run on DVE in parallel with Square.
                s = slice(c_lo, c_hi)
                nc.scalar.activation(out=d_[:, s], in_=ssq[:, s], func=Act.Sqrt)
                nc.vector.tensor_scalar(mask[:, s], d_[:, s], CUTOFF, None, Alu.is_lt)
                nc.vector.tensor_scalar(
                    fac[:, s], mask[:, s], PRESSURE - 1.0, 1.0, Alu.mult, Alu.add
                )
                nc.scalar.activation(
                    out=dd[:, s], in_=d_[:, s], func=Act.Square, bias=neg_cut[:]
                )
                nc.vector.tensor_tensor(
                    out=pen[:, s], in0=dd[:, s], in1=fac[:, s], op=Alu.mult
                )
                nc.vector.tensor_reduce(
                    out=psum[:, out_col:out_col + 1], in_=pen[:, s],
                    axis=mybir.AxisListType.X, op=Alu.add,
                )
                nc.sync.dma_start(
                    out=out[:, out_col:out_col + 1],
                    in_=psum[:, out_col:out_col + 1],
                )

            if not compute:
                nc.vector.memset(psum, 0.0)
                nc.sync.dma_start(out=out, in_=psum)
            for _r in range(repeat):
                if dma_group > 1 and not interleave:
                    # batched issue order: dma_group tiles' a-transfers
                    # back-to-back, then their b-transfers, then compute.
                    # Gives each input stream longer sequential runs per
                    # DMA queue.
                    r0 = 0
                    c0 = 0
                    descs = []
                    for kt in schedule:
                        descs.append((r0, c0, kt))
                        r0 += P * kt
                        c0 += kt
                    emitted_bulk = False
                    for g0 in range(0, len(descs), dma_group):
                        grp = descs[g0:g0 + dma_group]
                        tas, tbs = [], []
                        for (r0, c0, kt) in grp:
                            a_v = a[r0:r0 + P * kt, :].rearrange(
                                "(p k) d -> p (k d)", p=P)
                            ta = pa.tile([P, kt * D], f32, tag="ta")
                            nc.sync.dma_start(out=ta, in_=a_v)
                            tas.append(ta)
                        for (r0, c0, kt) in grp:
                            b_v = b[r0:r0 + P * kt, :].rearrange(
                                "(p k) d -> p (k d)", p=P)
                            tb = pb.tile([P, kt * D], f32, tag="tb")
                            nc.sync.dma_start(out=tb, in_=b_v)
                            tbs.append(tb)
                        if not compute:
                            continue
                        for i, (r0, c0, kt) in enumerate(grp):
                            ta, tb = tas[i], tbs[i]
                            nc.vector.tensor_tensor(out=ta, in0=ta, in1=tb,
                                                    op=Alu.subtract)
                            nc.scalar.activation(out=ta, in_=ta, func=Act.Square)
                            nc.vector.tensor_reduce(
                                out=ssq[:, c0:c0 + kt],
                                in_=ta.rearrange("p (k d) -> p k d", d=D),
                                axis=mybir.AxisListType.X,
                                op=Alu.add,
                            )
                            if (not emitted_bulk and 0 < split < T_units
                                    and c0 + kt >= split):
                                penalty_ops(0, split, 0)
                                emitted_bulk = True
                    continue
                r0 = 0   # row offset within the shard
                c0 = 0   # column offset within ssq
                for kt in schedule:
                    if interleave:
                        # one contiguous 2*kt*1KB-per-partition transfer from
                        # the single sequential stream
                        z_v = z[2 * r0:2 * r0 + 2 * P * kt, :].rearrange(
                            "(p k) d -> p (k d)", p=P
                        )
                        tz = pa.tile([P, 2 * kt * D], f32, tag="tz")
                        nc.sync.dma_start(out=tz, in_=z_v)
                        ta = tz[:, :kt * D]
                        tb = tz[:, kt * D:]
                    else:
                        # partition p holds kt consecutive rows -> contiguous
                        # kt*1KB per partition
                        a_v = a[r0:r0 + P * kt, :].rearrange("(p k) d -> p (k d)", p=P)
                        b_v = b[r0:r0 + P * kt, :].rearrange("(p k) d -> p (k d)", p=P)
                        ta = pa.tile([P, kt * D], f32, tag="ta")
                        tb = pb.tile([P, kt * D], f32, tag="tb")
                        nc.sync.dma_start(out=ta, in_=a_v)
                        # split_queues: b-stream on GpSimd SWDGE rings ->
                        # doubles the concurrent DMA queue set
                        (nc.gpsimd if split_queues else nc.sync).dma_start(
                            out=tb, in_=b_v)
                    r0 += P * kt
                    if not compute:
                        c0 += kt
                        continue
                    nc.vector.tensor_tensor(out=ta, in0=ta, in1=tb, op=Alu.subtract)
                    nc.scalar.activation(out=ta, in_=ta, func=Act.Square)
                    nc.vector.tensor_reduce(
                        out=ssq[:, c0:c0 + kt],
                        in_=ta.rearrange("p (k d) -> p k d", d=D),
                        axis=mybir.AxisListType.X,
                        op=Alu.add,
                    )
                    c0 += kt
                    if c0 == split and 0 < split < T_units:
                        # bulk penalty math, hidden under the taper tiles
                        penalty_ops(0, split, 0)

            if compute:
                if split == T_units:
                    penalty_ops(0, T_units, 0)
                else:
                    penalty_ops(split, T_units, 1)

    nc.compile()
    return nc


def interleave_inputs(a, b, schedule=None):
    """Host-side layout for interleave=True kernels: per tile, per partition,
    kt a-rows then kt b-rows, forming one sequential DRAM stream."""
    if schedule is None:
        schedule = K_DEFAULT
    b_local = a.shape[0]
    z = np.empty((2 * b_local, D), np.float32)
    r0 = 0
    for kt in schedule:
        rows = P * kt
        blk = z[2 * r0:2 * (r0 + rows)].reshape(P, 2 * kt, D)
        blk[:, :kt] = a[r0:r0 + rows].reshape(P, kt, D)
        blk[:, kt:] = b[r0:r0 + rows].reshape(P, kt, D)
        r0 += rows
    return z


_NC_CACHE = {}


def _get_nc():
    key = "default"
    if key not in _NC_CACHE:
        _NC_CACHE[key] = build_nc(b_local=B_LOCAL, k=K_DEFAULT, bufs=BUFS_DEFAULT)
    return _NC_CACHE[key]


def run_spmd(latent1, latent2, trace=False, **kwargs):
    """Shard inputs, run on 8 cores, return (scalar_loss, BassKernelResults)."""
    from concourse.bass_utils import run_bass_kernel_spmd

    nc = _get_nc()
    a = np.ascontiguousarray(np.asarray(latent1, dtype=np.float32))
    b = np.ascontiguousarray(np.asarray(latent2, dtype=np.float32))
    assert a.shape == (B, D) and b.shape == (B, D)
    in_maps = [
        {
            "latent1": a[c * B_LOCAL:(c + 1) * B_LOCAL],
            "latent2": b[c * B_LOCAL:(c + 1) * B_LOCAL],
        }
        for c in range(N_CORES)
    ]
    res = run_bass_kernel_spmd(
        nc, in_maps, core_ids=list(range(N_CORES)), trace=trace, **kwargs
    )
    total = sum(np.asarray(r["out"], dtype=np.float64).sum() for r in res.results)
    return np.asarray(total / B, dtype=np.float32), res


def kernel(latent1, latent2):
    loss, _ = run_spmd(latent1, latent2)
    return loss



# revision 12
# speedup vs baseline: 1.1118x; 1.1118x over previous
"""Trainium2 Bass kernel for EuclideanDistLoss.

reference:
    diff = latent1 - latent2                  # [B, D]
    d = sqrt(sum(diff^2, axis=1))             # [B]
    dev = d - CUTOFF
    penalty = where(dev > 0, dev^2, PRESSURE * dev^2)
    return mean(penalty)

Strategy: data-parallel over the batch dim across 8 NeuronCores. Each core
streams its 32768x256 shard of both inputs through SBUF ([128, k*256] tiles,
k rows per partition). Per tile: DVE subtract, then per 256-col group an ACT
Square with accum_out summing the group into one ssq column (mode="acc"), so
no DVE reduce exists and no engine ping-pongs: steady state is DMA-bound at
the ~358 GB/s/core HBM limit. A short penalty chain (Sqrt -> mask -> Square
-> mult -> reduce) runs mostly hidden under the stream; the host sums the
8x128x2 partials in float64 and divides by the global batch.

The earlier (v0) structure serialized sub(DVE) -> square(ACT) -> reduce(DVE)
per tile; in-order DVE made reduce(t) block sub(t+1), a 3.29us/tile chain vs
2.91us/tile of DMA. mode="pipe" fixes that by emitting reduce(t-1) after
sub(t); mode="acc" removes the DVE reduce entirely. A dummy Sqrt at program
start hoists the one-time ACT table load (1.3us) off the critical tail.
"""

import numpy as np

B, D = 262144, 256
N_CORES = 8
P = 128
CUTOFF = 0.1
PRESSURE = 10.0

B_LOCAL = B // N_CORES  # 32768
# per-tile schedule (rows per partition): bulk k=4 tiles, tapered end so the
# serial chain after the last transfer is short.
K_DEFAULT = [4] * 61 + [2] * 4 + [1] * 4
BUFS_DEFAULT = 8
TAIL_UNITS = 4          # columns handled in the post-stream tail chain
MODE_DEFAULT = "acc"
DVE_TAIL_KMAX = 2       # hyb: tiles with kt <= this run sub+sq+red all on DVE
ACC_TILES = 53          # mix: tiles [0, ACC_TILES) use ACT accum reduction;
                        # the rest use whole-tile square + deferred DVE reduce
                        # so ACT's accum backlog drains before the stream ends


def build_nc(b_local=B_LOCAL, k=K_DEFAULT, repeat=1, bufs=BUFS_DEFAULT,
             compute=True, mode=MODE_DEFAULT, tail_units=TAIL_UNITS,
             acc_tiles=ACC_TILES):
    """Build + compile the per-core Bass program (SPMD: same program on all
    cores).

    repeat>1 re-runs the streaming pass over the same data (benchmarking:
    slope of time vs repeat isolates pure on-device time). compute=False
    builds a DMA-only variant (bandwidth ceiling probe). mode: "acc" (ACT
    accum_out reduction), "pipe" (DVE reduce, software-pipelined), "v0"
    (original serialized chain).
    """
    import concourse.bacc as bacc
    import concourse.tile as tile
    from concourse import mybir

    f32 = mybir.dt.float32
    Alu = mybir.AluOpType
    Act = mybir.ActivationFunctionType

    if isinstance(k, int):
        tile_rows = P * k
        assert b_local % tile_rows == 0
        schedule = [k] * (b_local // tile_rows)
    else:
        schedule = list(k)
        assert sum(schedule) * P == b_local
    T_units = sum(schedule)  # total k-units (= ssq columns per partition)

    # columns [0, split) get their penalty math + partial-sum DMA issued while
    # the end of the stream is still in flight; [split, T) is the short tail.
    split = max(T_units - tail_units, 0) if (compute and repeat == 1) else T_units
    if mode == "flow":
        # chunked penalty: emit the penalty chain every CHUNK covered columns
        # so ACT absorbs it gradually; each chunk sums into its own psum col.
        CHUNK = 32
        chunk_bounds = list(range(CHUNK, T_units, CHUNK)) + [T_units]
        n_out_cols = len(chunk_bounds)
    else:
        n_out_cols = 2

    nc = bacc.Bacc("TRN2", target_bir_lowering=False, debug=False,
                   num_devices=N_CORES)
    a = nc.dram_tensor("latent1", [b_local, D], f32, kind="ExternalInput").ap()
    b = nc.dram_tensor("latent2", [b_local, D], f32, kind="ExternalInput").ap()
    out = nc.dram_tensor("out", [P, n_out_cols], f32, kind="ExternalOutput").ap()

    with tile.TileContext(nc) as tc:
        with (
            tc.tile_pool(name="pa", bufs=bufs) as pa,
            tc.tile_pool(name="pb", bufs=bufs) as pb,
            tc.tile_pool(name="keep", bufs=1) as keep,
        ):
            n = T_units
            ssq = keep.tile([P, n], f32)
            d_ = keep.tile([P, n], f32)
            mask = keep.tile([P, n], f32)  # 1.0 where d < CUTOFF
            fac = keep.tile([P, n], f32)   # 1 + (PRESSURE-1)*mask
            dd = keep.tile([P, n], f32)    # (d - CUTOFF)^2
            pen = keep.tile([P, n], f32)
            psum = keep.tile([P, n_out_cols], f32)
            neg_cut = keep.tile([P, 1], f32)
            warm = keep.tile([P, 1], f32)
            nc.vector.memset(neg_cut, -CUTOFF)
            # Dummy Sqrt: forces the one-time switch to the sqrt-capable ACT
            # table set during the DMA ramp instead of on the tail.
            nc.vector.memset(warm, 0.25)
            nc.scalar.activation(out=warm, in_=warm, func=Act.Sqrt)

            def penalty_ops(c_lo, c_hi, out_col):
                # critical path: Sqrt -> Square (same table set) -> mult ->
                # reduce; mask/fac run on DVE in parallel with Square. The
                # psum DMA issues from the ACT HWDGE ring so it never queues
                # ahead of remaining input-stream DMAs on the SP ring.
                s = slice(c_lo, c_hi)
                nc.scalar.activation(out=d_[:, s], in_=ssq[:, s], func=Act.Sqrt)
                nc.vector.tensor_scalar(mask[:, s], d_[:, s], CUTOFF, None,
                                        Alu.is_lt)
                nc.vector.tensor_scalar(
                    fac[:, s], mask[:, s], PRESSURE - 1.0, 1.0, Alu.mult, Alu.add
                )
                nc.scalar.activation(
                    out=dd[:, s], in_=d_[:, s], func=Act.Square, bias=neg_cut[:]
                )
                nc.vector.tensor_tensor(
                    out=pen[:, s], in0=dd[:, s], in1=fac[:, s], op=Alu.mult
                )
                nc.vector.tensor_reduce(
                    out=psum[:, out_col:out_col + 1], in_=pen[:, s],
                    axis=mybir.AxisListType.X, op=Alu.add,
                )
                nc.scalar.dma_start(
                    out=out[:, out_col:out_col + 1],
                    in_=psum[:, out_col:out_col + 1],
                )

            if not compute:
                nc.vector.memset(psum, 0.0)
                nc.sync.dma_start(out=out, in_=psum)
            def tile_style(idx, kt):
                if mode == "acc":
                    return "A"
                if mode == "pipe":
                    return "P"
                if mode == "v0":
                    return "V"
                if mode == "hyb":
                    return "D" if kt <= DVE_TAIL_KMAX else "A"
                if mode == "mix":
                    return "A" if idx < acc_tiles else "P"
                assert mode == "mix2"
                # acc everywhere; a short pipe block just before the taper
                # drains ACT's accum backlog so the taper's acc squares (and
                # the tail chain behind them) start with an idle ACT engine.
                return "P" if acc_tiles <= idx < acc_tiles + 4 else "A"

            for _r in range(repeat):
                r0 = 0   # row offset within the shard
                c0 = 0   # column offset within ssq
                covered = 0          # ssq columns whose producer is emitted
                pending_red = None   # style "P": deferred reduce descriptor
                emitted_bulk = False

                def flush_red():
                    nonlocal pending_red, covered
                    if pending_red is None:
                        return
                    pt, pc, pk = pending_red
                    nc.vector.tensor_reduce(
                        out=ssq[:, pc:pc + pk],
                        in_=pt.rearrange("p (k d) -> p k d", d=D),
                        axis=mybir.AxisListType.X, op=Alu.add,
                    )
                    pending_red = None
                    covered = pc + pk

                def maybe_bulk():
                    nonlocal emitted_bulk
                    if (not emitted_bulk and 0 < split < T_units
                            and covered >= split):
                        penalty_ops(0, split, 0)
                        emitted_bulk = True

                for idx, kt in enumerate(schedule):
                    # partition p holds kt consecutive rows -> contiguous
                    # kt*1KB per partition
                    a_v = a[r0:r0 + P * kt, :].rearrange("(p k) d -> p (k d)", p=P)
                    b_v = b[r0:r0 + P * kt, :].rearrange("(p k) d -> p (k d)", p=P)
                    ta = pa.tile([P, kt * D], f32, tag="ta")
                    tb = pb.tile([P, kt * D], f32, tag="tb")
                    nc.sync.dma_start(out=ta, in_=a_v)
                    nc.sync.dma_start(out=tb, in_=b_v)
                    r0 += P * kt
                    if not compute:
                        c0 += kt
                        continue
                    style = tile_style(idx, kt)
                    nc.vector.tensor_tensor(out=ta, in0=ta, in1=tb,
                                            op=Alu.subtract)
                    if style == "D":
                        # square + grouped reduce on DVE
                        nc.vector.tensor_tensor(out=ta, in0=ta, in1=ta,
                                                op=Alu.mult)
                        nc.vector.tensor_reduce(
                            out=ssq[:, c0:c0 + kt],
                            in_=ta.rearrange("p (k d) -> p k d", d=D),
                            axis=mybir.AxisListType.X, op=Alu.add,
                        )
                        covered = c0 + kt
                    elif style == "A":
                        for j in range(kt):
                            s = slice(j * D, (j + 1) * D)
                            nc.scalar.activation(
                                out=ta[:, s], in_=ta[:, s], func=Act.Square,
                                accum_out=ssq[:, c0 + j:c0 + j + 1],
                            )
                        covered = c0 + kt
                    elif style == "P":
                        flush_red()
                        nc.scalar.activation(out=ta, in_=ta, func=Act.Square)
                        pending_red = (ta, c0, kt)
                    else:  # "V"
                        nc.scalar.activation(out=ta, in_=ta, func=Act.Square)
                        nc.vector.tensor_reduce(
                            out=ssq[:, c0:c0 + kt],
                            in_=ta.rearrange("p (k d) -> p k d", d=D),
                            axis=mybir.AxisListType.X, op=Alu.add,
                        )
                        covered = c0 + kt
                    c0 += kt
                    maybe_bulk()
                flush_red()
                maybe_bulk()

            if compute:
                if split == T_units:
                    penalty_ops(0, T_units, 0)
                else:
                    penalty_ops(split, T_units, 1)

    nc.compile()
    return nc


_NC_CACHE = {}


def _get_nc():
    key = "default"
    if key not in _NC_CACHE:
        _NC_CACHE[key] = build_nc()
    return _NC_CACHE[key]


def run_spmd(latent1, latent2, trace=False, **kwargs):
    """Shard inputs, run on 8 cores, return (scalar_loss, BassKernelResults)."""
    from concourse.bass_utils import run_bass_kernel_spmd

    nc = _get_nc()
    a = np.ascontiguousarray(np.asarray(latent1, dtype=np.float32))
    b = np.ascontiguousarray(np.asarray(latent2, dtype=np.float32))
    assert a.shape == (B, D) and b.shape == (B, D)
    in_maps = [
        {
            "latent1": a[c * B_LOCAL:(c + 1) * B_LOCAL],
            "latent2": b[c * B_LOCAL:(c + 1) * B_LOCAL],
        }
        for c in range(N_CORES)
    ]
    res = run_bass_kernel_spmd(
        nc, in_maps, core_ids=list(range(N_CORES)), trace=trace, **kwargs
    )
    total = sum(np.asarray(r["out"], dtype=np.float64).sum() for r in res.results)
    return np.asarray(total / B, dtype=np.float32), res


def kernel(latent1, latent2):
    loss, _ = run_spmd(latent1, latent2)
    return loss


# revision 25
# speedup vs baseline: 1.2204x; 1.0977x over previous
"""Trainium2 Bass kernel for EuclideanDistLoss.

reference:
    diff = latent1 - latent2                  # [B, D]
    d = sqrt(sum(diff^2, axis=1))             # [B]
    dev = d - CUTOFF
    penalty = where(dev > 0, dev^2, PRESSURE * dev^2)
    return mean(penalty)

Strategy: data-parallel over the batch dim across 8 NeuronCores. Each core
streams its 32768x256 shard of both inputs through SBUF ([128, k*256] tiles,
k rows per partition). Per tile: DVE subtract, then per 256-col group an ACT
Square with accum_out summing the group into one ssq column (mode="acc"), so
no DVE reduce exists and no engine ping-pongs: steady state is DMA-bound at
the ~358 GB/s/core HBM limit. A short penalty chain (Sqrt -> mask -> Square
-> mult -> reduce) runs mostly hidden under the stream; the host sums the
8x128x2 partials in float64 and divides by the global batch.

The earlier (v0) structure serialized sub(DVE) -> square(ACT) -> reduce(DVE)
per tile; in-order DVE made reduce(t) block sub(t+1), a 3.29us/tile chain vs
2.91us/tile of DMA. mode="pipe" fixes that by emitting reduce(t-1) after
sub(t); mode="acc" removes the DVE reduce entirely. A dummy Sqrt at program
start hoists the one-time ACT table load (1.3us) off the critical tail.
"""

import numpy as np

B, D = 262144, 256
N_CORES = 8
P = 128
CUTOFF = 0.1
PRESSURE = 10.0

B_LOCAL = B // N_CORES  # 32768
# per-tile schedule (rows per partition): bulk k=4 tiles, tapered end so the
# serial chain after the last transfer is short. No k=1 tiles: two DMAs per
# 728ns of stream would exceed the HWDGE ring's ~625ns/DMA descriptor rate.
K_DEFAULT = [4] * 61 + [2] * 6
BUFS_DEFAULT = 16
TAIL_UNITS = 4          # columns handled in the post-stream tail chain
MODE_DEFAULT = "flow"
DVE_TAIL_KMAX = 2       # hyb: tiles with kt <= this run sub+sq+red all on DVE
ACC_TILES = 53          # mix: tiles [0, ACC_TILES) use ACT accum reduction;
                        # the rest use whole-tile square + deferred DVE reduce
                        # so ACT's accum backlog drains before the stream ends


def build_nc(b_local=B_LOCAL, k=K_DEFAULT, repeat=1, bufs=BUFS_DEFAULT,
             compute=True, mode=MODE_DEFAULT, tail_units=TAIL_UNITS,
             acc_tiles=ACC_TILES, b_ring="sync", chunk=32, dve_every=3):
    """Build + compile the per-core Bass program (SPMD: same program on all
    cores).

    repeat>1 re-runs the streaming pass over the same data (benchmarking:
    slope of time vs repeat isolates pure on-device time). compute=False
    builds a DMA-only variant (bandwidth ceiling probe). mode: "acc" (ACT
    accum_out reduction), "pipe" (DVE reduce, software-pipelined), "v0"
    (original serialized chain).
    """
    import concourse.bacc as bacc
    import concourse.tile as tile
    from concourse import mybir

    f32 = mybir.dt.float32
    Alu = mybir.AluOpType
    Act = mybir.ActivationFunctionType

    if isinstance(k, int):
        tile_rows = P * k
        assert b_local % tile_rows == 0
        schedule = [k] * (b_local // tile_rows)
    else:
        schedule = list(k)
        assert sum(schedule) * P == b_local
    T_units = sum(schedule)  # total k-units (= ssq columns per partition)

    # columns [0, split) get their penalty math + partial-sum DMA issued while
    # the end of the stream is still in flight; [split, T) is the short tail.
    split = max(T_units - tail_units, 0) if (compute and repeat == 1) else T_units
    if mode == "flow":
        # chunked penalty: emit the penalty chain every CHUNK covered columns
        # so ACT absorbs it gradually; each chunk sums into its own psum col.
        CHUNK = chunk
        chunk_bounds = list(range(CHUNK, T_units, CHUNK)) + [T_units]
        n_out_cols = len(chunk_bounds)
    else:
        n_out_cols = 2

    nc = bacc.Bacc("TRN2", target_bir_lowering=False, debug=False,
                   num_devices=N_CORES)
    a = nc.dram_tensor("latent1", [b_local, D], f32, kind="ExternalInput").ap()
    b = nc.dram_tensor("latent2", [b_local, D], f32, kind="ExternalInput").ap()
    out = nc.dram_tensor("out", [P, n_out_cols], f32, kind="ExternalOutput").ap()

    with tile.TileContext(nc) as tc:
        with (
            tc.tile_pool(name="pa", bufs=bufs) as pa,
            tc.tile_pool(name="pb", bufs=bufs) as pb,
            tc.tile_pool(name="keep", bufs=1) as keep,
        ):
            n = T_units
            ssq = keep.tile([P, n], f32)
            d_ = keep.tile([P, n], f32)
            mask = keep.tile([P, n], f32)  # 1.0 where d < CUTOFF
            fac = keep.tile([P, n], f32)   # 1 + (PRESSURE-1)*mask
            dd = keep.tile([P, n], f32)    # (d - CUTOFF)^2
            pen = keep.tile([P, n], f32)
            psum = keep.tile([P, n_out_cols], f32)
            neg_cut = keep.tile([P, 1], f32)
            warm = keep.tile([P, 1], f32)
            nc.vector.memset(neg_cut, -CUTOFF)
            # Dummy Sqrt: forces the one-time switch to the sqrt-capable ACT
            # table set during the DMA ramp instead of on the tail.
            nc.vector.memset(warm, 0.25)
            nc.scalar.activation(out=warm, in_=warm, func=Act.Sqrt)

            def penalty_ops(c_lo, c_hi, out_col):
                # critical path: Sqrt -> Square (same table set) -> mult ->
                # reduce; mask/fac run on DVE in parallel with Square. The
                # psum DMA issues from the ACT HWDGE ring so it never queues
                # ahead of remaining input-stream DMAs on the SP ring.
                s = slice(c_lo, c_hi)
                nc.scalar.activation(out=d_[:, s], in_=ssq[:, s], func=Act.Sqrt)
                nc.vector.tensor_scalar(mask[:, s], d_[:, s], CUTOFF, None,
                                        Alu.is_lt)
                nc.vector.tensor_scalar(
                    fac[:, s], mask[:, s], PRESSURE - 1.0, 1.0, Alu.mult, Alu.add
                )
                nc.scalar.activation(
                    out=dd[:, s], in_=d_[:, s], func=Act.Square, bias=neg_cut[:]
                )
                nc.vector.tensor_tensor(
                    out=pen[:, s], in0=dd[:, s], in1=fac[:, s], op=Alu.mult
                )
                nc.vector.tensor_reduce(
                    out=psum[:, out_col:out_col + 1], in_=pen[:, s],
                    axis=mybir.AxisListType.X, op=Alu.add,
                )
                nc.scalar.dma_start(
                    out=out[:, out_col:out_col + 1],
                    in_=psum[:, out_col:out_col + 1],
                )

            if not compute:
                nc.vector.memset(psum, 0.0)
                nc.sync.dma_start(out=out, in_=psum)
            def tile_style(idx, kt):
                if mode in ("acc", "flow"):
                    return "A"
                if mode == "pipe":
                    return "P"
                if mode == "v0":
                    return "V"
                if mode == "hyb":
                    return "D" if kt <= DVE_TAIL_KMAX else "A"
                if mode == "mix":
                    return "A" if idx < acc_tiles else "P"
                assert mode == "mix2"
                # acc everywhere; a short pipe block just before the taper
                # drains ACT's accum backlog so the taper's acc squares (and
                # the tail chain behind them) start with an idle ACT engine.
                return "P" if acc_tiles <= idx < acc_tiles + 4 else "A"

            for _r in range(repeat):
                r0 = 0   # row offset within the shard
                c0 = 0   # column offset within ssq
                covered = 0          # ssq columns whose producer is emitted
                pending_red = None   # style "P": deferred reduce descriptor
                emitted_bulk = False
                next_chunk = 0       # mode "flow": next penalty chunk to emit

                def flush_red():
                    nonlocal pending_red, covered
                    if pending_red is None:
                        return
                    pt, pc, pk = pending_red
                    nc.vector.tensor_reduce(
                        out=ssq[:, pc:pc + pk],
                        in_=pt.rearrange("p (k d) -> p k d", d=D),
                        axis=mybir.AxisListType.X, op=Alu.add,
                    )
                    pending_red = None
                    covered = pc + pk

                def maybe_bulk():
                    nonlocal emitted_bulk, next_chunk
                    if mode == "flow":
                        while (next_chunk < len(chunk_bounds)
                               and covered >= chunk_bounds[next_chunk]):
                            lo = chunk_bounds[next_chunk - 1] if next_chunk else 0
                            penalty_ops(lo, chunk_bounds[next_chunk], next_chunk)
                            next_chunk += 1
                        return
                    if (not emitted_bulk and 0 < split < T_units
                            and covered >= split):
                        penalty_ops(0, split, 0)
                        emitted_bulk = True

                for idx, kt in enumerate(schedule):
                    # partition p holds kt consecutive rows -> contiguous
                    # kt*1KB per partition
                    a_v = a[r0:r0 + P * kt, :].rearrange("(p k) d -> p (k d)", p=P)
                    b_v = b[r0:r0 + P * kt, :].rearrange("(p k) d -> p (k d)", p=P)
                    ta = pa.tile([P, kt * D], f32, tag="ta")
                    tb = pb.tile([P, kt * D], f32, tag="tb")
                    nc.sync.dma_start(out=ta, in_=a_v)
                    # b-stream on a second descriptor-generation ring: one
                    # HWDGE ring (625ns/DMA desc-gen) can't keep up with two
                    # DMAs per small taper tile
                    getattr(nc, b_ring).dma_start(out=tb, in_=b_v)
                    r0 += P * kt
                    if not compute:
                        c0 += kt
                        continue
                    style = tile_style(idx, kt)
                    if mode == "flow":
                        # unit-granularity: sub_j then square+accum_j, so ACT
                        # units start 327ns (not 1127ns) after each DMA and
                        # the pipeline latency stays ~0.9us the whole stream
                        for j in range(kt):
                            s = slice(j * D, (j + 1) * D)
                            u = c0 + j
                            nc.vector.tensor_tensor(out=ta[:, s], in0=ta[:, s],
                                                    in1=tb[:, s],
                                                    op=Alu.subtract)
                            if dve_every and (u % dve_every == dve_every - 1):
                                # spread reduction load: this unit squares and
                                # reduces on DVE instead of ACT
                                nc.vector.tensor_tensor(
                                    out=ta[:, s], in0=ta[:, s], in1=ta[:, s],
                                    op=Alu.mult)
                                nc.vector.tensor_reduce(
                                    out=ssq[:, u:u + 1], in_=ta[:, s],
                                    axis=mybir.AxisListType.X, op=Alu.add)
                            else:
                                nc.scalar.activation(
                                    out=ta[:, s], in_=ta[:, s], func=Act.Square,
                                    accum_out=ssq[:, u:u + 1],
                                )
                            covered = u + 1
                            maybe_bulk()
                        c0 += kt
                        continue
                    nc.vector.tensor_tensor(out=ta, in0=ta, in1=tb,
                                            op=Alu.subtract)
                    if style == "D":
                        # square + grouped reduce on DVE
                        nc.vector.tensor_tensor(out=ta, in0=ta, in1=ta,
                                                op=Alu.mult)
                        nc.vector.tensor_reduce(
                            out=ssq[:, c0:c0 + kt],
                            in_=ta.rearrange("p (k d) -> p k d", d=D),
                            axis=mybir.AxisListType.X, op=Alu.add,
                        )
                        covered = c0 + kt
                    elif style == "A":
                        for j in range(kt):
                            s = slice(j * D, (j + 1) * D)
                            nc.scalar.activation(
                                out=ta[:, s], in_=ta[:, s], func=Act.Square,
                                accum_out=ssq[:, c0 + j:c0 + j + 1],
                            )
                        covered = c0 + kt
                    elif style == "P":
                        flush_red()
                        nc.scalar.activation(out=ta, in_=ta, func=Act.Square)
                        pending_red = (ta, c0, kt)
                    else:  # "V"
                        nc.scalar.activation(out=ta, in_=ta, func=Act.Square)
                        nc.vector.tensor_reduce(
                            out=ssq[:, c0:c0 + kt],
                            in_=ta.rearrange("p (k d) -> p k d", d=D),
                            axis=mybir.AxisListType.X, op=Alu.add,
                        )
                        covered = c0 + kt
                    c0 += kt
                    maybe_bulk()
                flush_red()
                maybe_bulk()

            if compute and mode != "flow":
                if split == T_units:
                    penalty_ops(0, T_units, 0)
                else:
                    penalty_ops(split, T_units, 1)

    nc.compile()
    return nc


_NC_CACHE = {}


def _get_nc():
    key = "default"
    if key not in _NC_CACHE:
        _NC_CACHE[key] = build_nc()
    return _NC_CACHE[key]


def run_spmd(latent1, latent2, trace=False, **kwargs):
    """Shard inputs, run on 8 cores, return (scalar_loss, BassKernelResults)."""
    from concourse.bass_utils import run_bass_kernel_spmd

    nc = _get_nc()
    a = np.ascontiguousarray(np.asarray(latent1, dtype=np.float32))
    b = np.ascontiguousarray(np.asarray(latent2, dtype=np.float32))
    assert a.shape == (B, D) and b.shape == (B, D)
    in_maps = [
        {
            "latent1": a[c * B_LOCAL:(c + 1) * B_LOCAL],
            "latent2": b[c * B_LOCAL:(c + 1) * B_LOCAL],
        }
        for c in range(N_CORES)
    ]
    res = run_bass_kernel_spmd(
        nc, in_maps, core_ids=list(range(N_CORES)), trace=trace, **kwargs
    )
    total = sum(np.asarray(r["out"], dtype=np.float64).sum() for r in res.results)
    return np.asarray(total / B, dtype=np.float32), res


def kernel(latent1, latent2):
    loss, _ = run_spmd(latent1, latent2)
    return loss
